# revision 42
# baseline (speedup 1.0000x reference)
"""MAE encoder (ViT-B-ish, 12 layers) on 8 Trainium2 NeuronCores.

Strategy: data-parallel over batch (16 images -> 2 per core), weights
replicated. Inside each core activations live in "feature-on-partition"
layout xT[128, E/128, T] so every projection is matmul(lhsT=W^T, rhs=xT)
with no transposes. Attention computes S^T = k_h^T.T @ q_h^T directly
(keys on partitions), softmax normalizer via a ones-matmul that lands
replicated across partitions, so no transpose is ever needed.
Matmul inputs bf16, residual stream fp32.
"""

import numpy as np
import ml_dtypes

import concourse.bass as bass
import concourse.bacc as bacc
import concourse.mybir as mybir
import concourse.tile as tile
from concourse.bass_utils import run_bass_kernel_spmd

BF16 = mybir.dt.bfloat16
F32 = mybir.dt.float32
F8 = mybir.dt.float8e4
DR = mybir.MatmulPerfMode.DoubleRow
nf8 = mybir.dt.np(F8)
AF = mybir.ActivationFunctionType
OP = mybir.AluOpType

B, C, IMG, P = 16, 1, 512, 16
GRID = IMG // P
N = GRID * GRID          # 1024
E = 768
H = 12
HD = 64
FF = 4 * E               # 3072
NUM_KEEP = N // 2        # 512
LN_EPS = 1e-5
NCORES = 8
IPC = B // NCORES        # images per core = 2
T = IPC * NUM_KEEP       # tokens per core = 1024
TI = NUM_KEEP            # tokens per image = 512
EC = E // 128            # 6 feature chunks
FC = FF // 128           # 24 hidden chunks
PD = C * P * P           # 256 patch dim
KC = PD // 128           # 2

nbf = ml_dtypes.bfloat16

# lnp param tensor column layout (per layer, [128, 72] f32)
LN1W, LN1B, QKB, PRB, LN2W, LN2B, F1B, F2B = 0, 6, 12, 24, 30, 36, 42, 66


def build_nc(depth):
    nc = bacc.Bacc(None, target_bir_lowering=False)
    d = {}
    d['patches'] = nc.dram_tensor('patches', [128, KC, T], BF16, kind='ExternalInput')
    d['posT'] = nc.dram_tensor('posT', [128, EC, T], F32, kind='ExternalInput')
    d['pwT'] = nc.dram_tensor('pwT', [128, KC, E], BF16, kind='ExternalInput')
    d['pepar'] = nc.dram_tensor('pepar', [128, 3 * EC], F32, kind='ExternalInput')
    d['npar'] = nc.dram_tensor('npar', [128, 2 * EC], F32, kind='ExternalInput')
    d['wqk'] = nc.dram_tensor('wqk', [depth, 128, EC, 2 * E], BF16, kind='ExternalInput')
    d['wv'] = nc.dram_tensor('wv', [depth, 128, EC, E], BF16, kind='ExternalInput')
    d['wp'] = nc.dram_tensor('wp', [depth, 128, EC, E], BF16, kind='ExternalInput')
    d['w1'] = nc.dram_tensor('w1', [depth, 2, 128, EC, FF // 2], BF16, kind='ExternalInput')
    d['w2'] = nc.dram_tensor('w2', [depth, 2, 128, FC, E // 2], BF16, kind='ExternalInput')
    d['lnp'] = nc.dram_tensor('lnp', [depth, 128, 72], F32, kind='ExternalInput')
    d['vb'] = nc.dram_tensor('vb', [depth, 128, E], BF16, kind='ExternalInput')
    d['xvis'] = nc.dram_tensor('xvis', [128, EC, T], F32, kind='ExternalOutput')

    with tile.TileContext(nc) as tc:
        _emit(nc, tc, d, depth)
    nc.finalize()
    return nc


PHASE_MARKS = []


def _mark(nc, label):
    n = nc.next_id()   # consumes one id; fine, just a marker
    PHASE_MARKS.append((n, label))


def _emit(nc, tc, d, depth):
    PHASE_MARKS.clear()
    import contextlib
    ctx = contextlib.ExitStack()
    with ctx:
        wpool = ctx.enter_context(tc.tile_pool(name='w', bufs=2))
        state = ctx.enter_context(tc.tile_pool(name='state', bufs=1))
        hx = ctx.enter_context(tc.tile_pool(name='hx', bufs=3))
        actp = ctx.enter_context(tc.tile_pool(name='act', bufs=2))
        actg = ctx.enter_context(tc.tile_pool(name='actg', bufs=1))
        atp = ctx.enter_context(tc.tile_pool(name='at', bufs=4))
        stp = ctx.enter_context(tc.tile_pool(name='st', bufs=2))
        tmpp = ctx.enter_context(tc.tile_pool(name='tmp', bufs=2))
        smallp = ctx.enter_context(tc.tile_pool(name='small', bufs=2))
        outp = ctx.enter_context(tc.tile_pool(name='outp', bufs=1))
        ps = ctx.enter_context(tc.tile_pool(name='ps', bufs=6, space='PSUM'))
        pstat = ctx.enter_context(tc.tile_pool(name='pstat', bufs=2, space='PSUM'))

        # persistent state
        xres = state.tile([128, EC, T], F32)       # residual stream (both imgs)
        ones = state.tile([128, 128], BF16)
        nc.vector.memset(ones, 1.0)
        epst = state.tile([128, 1], F32)
        nc.vector.memset(epst, LN_EPS)

        def mm_chain(out_ap, lhsT_slices, rhs_slices, tile_position=None):
            n = len(lhsT_slices)
            for i in range(n):
                nc.tensor.matmul(out_ap, lhsT_slices[i], rhs_slices[i],
                                 start=(i == 0), stop=(i == n - 1),
                                 tile_position=tile_position)

        def ln_stats(xb, nchunks):
            """xb: [128, nchunks, TI] bf16. Returns (m_sb, rstd) [128, TI] f32."""
            s_ps = pstat.tile([128, TI], F32, tag='stat')
            sq_ps = pstat.tile([128, TI], F32, tag='stat')
            mm_chain(s_ps, [ones] * nchunks,
                     [xb[:, k, :] for k in range(nchunks)])
            for k in range(nchunks):
                xsq = tmpp.tile([128, TI], BF16, tag='xsq')
                nc.vector.tensor_mul(xsq, xb[:, k, :], xb[:, k, :])
                nc.tensor.matmul(sq_ps, ones, xsq, start=(k == 0), stop=(k == nchunks - 1))
            cinv = 1.0 / (nchunks * 128)
            m_sb = stp.tile([128, TI], F32, tag='m')
            nc.vector.tensor_scalar_mul(m_sb, s_ps, cinv)
            m2 = stp.tile([128, TI], F32, tag='m2')
            nc.vector.tensor_mul(m2, m_sb, m_sb)
            var = stp.tile([128, TI], F32, tag='var')
            nc.vector.scalar_tensor_tensor(var, sq_ps, cinv, m2,
                                           op0=OP.mult, op1=OP.subtract)
            # rstd = exp(-0.5*ln(var+eps)); Ln+Exp share one ACT table set
            nc.scalar.activation(var, var, AF.Ln, bias=epst)
            nc.scalar.activation(var, var, AF.Exp, scale=-0.5)
            return m_sb, var

        def ln_apply(xb, m_sb, rstd, w_col, b_col, out_tile, nchunks):
            """out[:,k,:] = ((xb[:,k,:]-m)*rstd)*w[k] + b[k]"""
            for k in range(nchunks):
                t0 = tmpp.tile([128, TI], BF16, tag='ta')
                nc.vector.tensor_sub(t0, xb[:, k, :], m_sb)
                t1 = tmpp.tile([128, TI], BF16, tag='tb')
                nc.vector.tensor_mul(t1, t0, rstd)
                nc.scalar.activation(out_tile[:, k, :], t1, AF.Identity,
                                     bias=b_col(k), scale=w_col(k))

        # ---------------- patch embed + pe-norm + pos ----------------
        _mark(nc, 'patch')
        pw = wpool.tile([128, KC, E], BF16, tag='w')
        nc.sync.dma_start(pw, d['pwT'][:])
        pep = smallp.tile([128, 3 * EC], F32, tag='pep')
        nc.sync.dma_start(pep, d['pepar'][:])
        for img in range(IPC):
            pt = wpool.tile([128, KC, TI], BF16, tag='w')
            nc.sync.dma_start(pt, d['patches'][:, :, img * TI:(img + 1) * TI])
            tb = hx.tile([128, EC, TI], BF16, tag='hx')  # embedded tokens bf16
            for ec in range(EC):
                pp = ps.tile([128, TI], F32, tag='mm')
                mm_chain(pp, [pw[:, k, ec * 128:(ec + 1) * 128] for k in range(KC)],
                         [pt[:, k, :] for k in range(KC)])
                nc.scalar.activation(tb[:, ec, :], pp, AF.Identity,
                                     bias=pep[:, ec:ec + 1])
            m_sb, rstd = ln_stats(tb, EC)
            for ec in range(EC):
                t0 = tmpp.tile([128, TI], BF16, tag='ta')
                nc.vector.tensor_sub(t0, tb[:, ec, :], m_sb)
                t1 = tmpp.tile([128, TI], BF16, tag='tb')
                nc.vector.tensor_mul(t1, t0, rstd)
                t2 = tmpp.tile([128, TI], F32, tag='tc')
                nc.scalar.activation(t2, t1, AF.Identity,
                                     bias=pep[:, 2 * EC + ec:2 * EC + ec + 1],
                                     scale=pep[:, EC + ec:EC + ec + 1])
                pos = outp.tile([128, TI], F32, tag='o')
                nc.sync.dma_start(pos, d['posT'][:, ec, img * TI:(img + 1) * TI])
                nc.vector.tensor_add(xres[:, ec, img * TI:(img + 1) * TI], t2, pos)

        # ---------------- transformer layers ----------------
        for l in range(depth):
            lnp = smallp.tile([128, 72], F32, tag='lnp')
            nc.sync.dma_start(lnp, d['lnp'][l])
            vb = smallp.tile([128, E], BF16, tag='vb')
            nc.sync.dma_start(vb, d['vb'][l])
            wqk = wpool.tile([128, EC, 2 * E], BF16, tag='w')
            nc.sync.dma_start(wqk, d['wqk'][l])
            wv = wpool.tile([128, EC, E], BF16, tag='w')
            nc.sync.dma_start(wv, d['wv'][l])

            qks, vs, ots = [], [], []
            # ---- phase A: LN1 + qkv (both images) ----
            for img in range(IPC):
                tsl = slice(img * TI, (img + 1) * TI)
                # ---- LN1 -> h ----
                _mark(nc, 'ln1')
                xb = hx.tile([128, EC, TI], BF16, tag='hx')
                for k in range(EC):
                    nc.vector.tensor_copy(xb[:, k, :], xres[:, k, tsl])
                m_sb, rstd = ln_stats(xb, EC)
                h = hx.tile([128, EC, TI], BF16, tag='hx')
                ln_apply(xb, m_sb, rstd,
                         lambda k: lnp[:, LN1W + k:LN1W + k + 1],
                         lambda k: lnp[:, LN1B + k:LN1B + k + 1], h, EC)

                # ---- qkv ----
                _mark(nc, 'qkv')
                qk = actp.tile([128, 2 * EC, TI], BF16, tag='qk')
                for mc in range(2 * EC):
                    pp = ps.tile([128, TI], F32, tag='mm')
                    mm_chain(pp, [wqk[:, k, mc * 128:(mc + 1) * 128] for k in range(EC)],
                             [h[:, k, :] for k in range(EC)])
                    nc.scalar.activation(qk[:, mc, :], pp, AF.Identity,
                                         bias=lnp[:, QKB + mc:QKB + mc + 1])
                v = actp.tile([128, TI // 128, E], BF16, tag='v')
                for tcx in range(TI // 128):
                    for nh in range(2):
                        nsl = slice(nh * (E // 2), (nh + 1) * (E // 2))
                        pp = ps.tile([128, E // 2], F32, tag='mm')
                        mm_chain(pp, [h[:, k, tcx * 128:(tcx + 1) * 128] for k in range(EC)],
                                 [wv[:, k, nsl] for k in range(EC)])
                        nc.vector.tensor_tensor(v[:, tcx, nsl], pp, vb[:, nsl], OP.add)
                qks.append(qk)
                vs.append(v)

            # ---- phase B: attention (both images) ----
            _mark(nc, 'attn')
            for img in range(IPC):
                qk, v = qks[img], vs[img]
                ot = actp.tile([128, EC, TI], BF16, tag='ot')
                for hd in range(H):
                    qc, ro = hd // 2, 64 * (hd % 2)
                    at = atp.tile([128, 4, TI], BF16, tag='at')
                    for jc in range(4):
                        sp = ps.tile([128, TI], F32, tag='mm')
                        nc.tensor.matmul(
                            sp,
                            qk[ro:ro + 64, EC + qc, jc * 128:(jc + 1) * 128],
                            qk[ro:ro + 64, qc, :],
                            start=True, stop=True)
                        nc.scalar.activation(at[:, jc, :], sp, AF.Exp)
                    op_t = ps.tile([128, TI], F32, tag='mm')
                    for jc in range(4):
                        nc.tensor.matmul(op_t[0:64, :],
                                         v[:, jc, hd * 64:hd * 64 + 64],
                                         at[:, jc, :],
                                         start=(jc == 0), stop=(jc == 3),
                                         tile_position=(0, 0))
                    for jc in range(4):
                        nc.tensor.matmul(op_t[64:128, :], ones[:, 0:64],
                                         at[:, jc, :],
                                         start=(jc == 0), stop=(jc == 3),
                                         tile_position=(0, 64))
                    r = stp.tile([64, TI], F32, tag='r')
                    nc.vector.reciprocal(r, op_t[64:128, :])
                    nc.vector.tensor_mul(ot[ro:ro + 64, qc, :],
                                         op_t[0:64, :], r)
                ots.append(ot)

            # ---- phase C: proj + residual + LN2 (both images) ----
            wp = wpool.tile([128, EC, E], BF16, tag='w')
            nc.sync.dma_start(wp, d['wp'][l])
            h2s = []
            for img in range(IPC):
                tsl = slice(img * TI, (img + 1) * TI)
                _mark(nc, 'proj')
                ot = ots[img]
                for ec in range(EC):
                    pp = ps.tile([128, TI], F32, tag='mm')
                    mm_chain(pp, [wp[:, k, ec * 128:(ec + 1) * 128] for k in range(EC)],
                             [ot[:, k, :] for k in range(EC)])
                    nc.vector.scalar_tensor_tensor(
                        xres[:, ec, tsl], pp, lnp[:, PRB + ec:PRB + ec + 1],
                        xres[:, ec, tsl], op0=OP.add, op1=OP.add)
                _mark(nc, 'ln2')
                xb2 = hx.tile([128, EC, TI], BF16, tag='hx')
                for k in range(EC):
                    nc.vector.tensor_copy(xb2[:, k, :], xres[:, k, tsl])
                m_sb2, rstd2 = ln_stats(xb2, EC)
                h2 = hx.tile([128, EC, TI], BF16, tag='hx')
                ln_apply(xb2, m_sb2, rstd2,
                         lambda k: lnp[:, LN2W + k:LN2W + k + 1],
                         lambda k: lnp[:, LN2B + k:LN2B + k + 1], h2, EC)
                h2s.append(h2)

            # ---- phase D: MLP (both images) ----
            _mark(nc, 'mlp')
            for img in range(IPC):
                tsl = slice(img * TI, (img + 1) * TI)
                h2 = h2s[img]
                g = actg.tile([128, FC, TI], BF16, tag='g')
                for half in range(2):
                    w1h = wpool.tile([128, EC, FF // 2], BF16, tag='w')
                    nc.sync.dma_start(w1h, d['w1'][l, half])
                    for hc in range(FC // 2):
                        ghc = half * (FC // 2) + hc
                        pp = ps.tile([128, TI], F32, tag='mm')
                        mm_chain(pp, [w1h[:, k, hc * 128:(hc + 1) * 128] for k in range(EC)],
                                 [h2[:, k, :] for k in range(EC)])
                        nc.scalar.activation(g[:, ghc, :], pp, AF.Gelu,
                                             bias=lnp[:, F1B + ghc:F1B + ghc + 1])
                for half in range(2):
                    w2h = wpool.tile([128, FC, E // 2], BF16, tag='w')
                    nc.sync.dma_start(w2h, d['w2'][l, half])
                    for ec in range(EC // 2):
                        oec = half * (EC // 2) + ec
                        pp = ps.tile([128, TI], F32, tag='mm')
                        mm_chain(pp, [w2h[:, k, ec * 128:(ec + 1) * 128] for k in range(FC)],
                                 [g[:, k, :] for k in range(FC)])
                        nc.vector.scalar_tensor_tensor(
                            xres[:, oec, tsl], pp, lnp[:, F2B + oec:F2B + oec + 1],
                            xres[:, oec, tsl], op0=OP.add, op1=OP.add)

        # ---------------- final norm + output ----------------
        _mark(nc, 'final')
        npar = smallp.tile([128, 2 * EC], F32, tag='npar')
        nc.sync.dma_start(npar, d['npar'][:])
        for img in range(IPC):
            tsl = slice(img * TI, (img + 1) * TI)
            xb = hx.tile([128, EC, TI], BF16, tag='hx')
            for k in range(EC):
                nc.vector.tensor_copy(xb[:, k, :], xres[:, k, tsl])
            m_sb, rstd = ln_stats(xb, EC)
            for ec in range(EC):
                t0 = tmpp.tile([128, TI], F32, tag='fa')
                nc.vector.tensor_sub(t0, xres[:, ec, tsl], m_sb)
                t1 = tmpp.tile([128, TI], F32, tag='fb')
                nc.vector.tensor_mul(t1, t0, rstd)
                o = outp.tile([128, TI], F32, tag='o')
                nc.scalar.activation(o, t1, AF.Identity,
                                     bias=npar[:, EC + ec:EC + ec + 1],
                                     scale=npar[:, ec:ec + 1])
                nc.sync.dma_start(d['xvis'][:, ec, tsl], o)


# ---------------------------------------------------------------------------
# host-side data prep
# ---------------------------------------------------------------------------

def _featT(W):
    """W [O, I] row-major -> [128, I//128, O] so tile[ki, ks, o] = W[o, ks*128+ki]."""
    O, I = W.shape
    return np.ascontiguousarray(W.T.reshape(I // 128, 128, O).transpose(1, 0, 2))


def _tokT(X):
    """X [T, E] -> [128, E//128, T] so tile[ki, ks, t] = X[t, ks*128+ki]."""
    Tn, En = X.shape
    return np.ascontiguousarray(X.T.reshape(En // 128, 128, Tn).transpose(1, 0, 2))


def _cols(vec):
    """[D] -> [128, D//128] so out[ki, k] = vec[k*128+ki]."""
    D = vec.shape[0]
    return np.ascontiguousarray(vec.reshape(D // 128, 128).T)


def prep_weights(inputs, depth):
    f32 = np.float32
    qkv_w = np.asarray(inputs['qkv_w'], f32).copy()
    qkv_b = np.asarray(inputs['qkv_b'], f32).copy()
    scale = HD ** -0.5
    qkv_w[:, :E, :] *= scale
    qkv_b[:, :E] *= scale

    wqk = np.stack([_featT(qkv_w[l, :2 * E, :]) for l in range(depth)]).astype(nbf)
    # v weights as rhs: [ki, ks, d] = Wv[d, ks*128+ki] = Wv.T[ks*128+ki, d]
    wv = np.stack([_featT(qkv_w[l, 2 * E:, :]) for l in range(depth)]).astype(nbf)
    wp = np.stack([_featT(np.asarray(inputs['proj_w'][l], f32)) for l in range(depth)]).astype(nbf)
    w1full = [_featT(np.asarray(inputs['fc1_w'][l], f32)) for l in range(depth)]
    w1 = np.stack([np.stack([w[:, :, :FF // 2], w[:, :, FF // 2:]]) for w in w1full]).astype(nbf)
    w2full = [_featT(np.asarray(inputs['fc2_w'][l], f32)) for l in range(depth)]
    w2 = np.stack([np.stack([w[:, :, :E // 2], w[:, :, E // 2:]]) for w in w2full]).astype(nbf)

    lnp = np.zeros((depth, 128, 72), f32)
    for l in range(depth):
        lnp[l, :, LN1W:LN1W + 6] = _cols(np.asarray(inputs['ln1_w'][l], f32))
        lnp[l, :, LN1B:LN1B + 6] = _cols(np.asarray(inputs['ln1_b'][l], f32))
        lnp[l, :, QKB:QKB + 12] = _cols(qkv_b[l, :2 * E])
        lnp[l, :, PRB:PRB + 6] = _cols(np.asarray(inputs['proj_b'][l], f32))
        lnp[l, :, LN2W:LN2W + 6] = _cols(np.asarray(inputs['ln2_w'][l], f32))
        lnp[l, :, LN2B:LN2B + 6] = _cols(np.asarray(inputs['ln2_b'][l], f32))
        lnp[l, :, F1B:F1B + 24] = _cols(np.asarray(inputs['fc1_b'][l], f32))
        lnp[l, :, F2B:F2B + 6] = _cols(np.asarray(inputs['fc2_b'][l], f32))
    vbr = np.broadcast_to(qkv_b[:depth, 2 * E:, None].transpose(0, 2, 1),
                          (depth, 128, E)).astype(nbf)
    vbr = np.ascontiguousarray(vbr)

    pwT = _featT(np.asarray(inputs['patch_w'], f32).reshape(E, PD)).astype(nbf)
    pepar = np.zeros((128, 18), f32)
    pepar[:, 0:6] = _cols(np.asarray(inputs['patch_b'], f32))
    pepar[:, 6:12] = _cols(np.asarray(inputs['pe_norm_w'], f32))
    pepar[:, 12:18] = _cols(np.asarray(inputs['pe_norm_b'], f32))
    npar = np.zeros((128, 12), f32)
    npar[:, 0:6] = _cols(np.asarray(inputs['norm_w'], f32))
    npar[:, 6:12] = _cols(np.asarray(inputs['norm_b'], f32))
    return dict(wqk=wqk, wv=wv, wp=wp, w1=w1, w2=w2, lnp=lnp, vb=vbr,
                pwT=pwT, pepar=pepar, npar=npar)


def prep_percore(inputs):
    f32 = np.float32
    x = np.asarray(inputs['x'], f32)
    noise = np.asarray(inputs['noise'], f32)
    ids_shuffle = np.argsort(noise, axis=1, kind='stable').astype(np.int32)
    ids_restore = np.argsort(ids_shuffle, axis=1, kind='stable').astype(np.int32)
    ids_keep = ids_shuffle[:, :NUM_KEEP]
    mask = np.ones((B, N), f32)
    mask[:, :NUM_KEEP] = 0.0
    mask = np.take_along_axis(mask, ids_restore, axis=1)

    patches = x.reshape(B, GRID, P, GRID, P).transpose(0, 1, 3, 2, 4).reshape(B, N, PD)
    pos = np.asarray(inputs['pos_embed'], f32)[0]        # [N, E]

    per_core = []
    for c in range(NCORES):
        sel = ids_keep[c * IPC:(c + 1) * IPC]            # [IPC, 512]
        pk = np.concatenate([patches[c * IPC + i][sel[i]] for i in range(IPC)])  # [T, PD]
        pc = _tokT(pk).astype(nbf)                        # [128, KC, T]
        pg = np.concatenate([pos[sel[i]] for i in range(IPC)])                   # [T, E]
        pT = _tokT(pg)                                    # [128, EC, T] f32
        per_core.append((np.ascontiguousarray(pc), np.ascontiguousarray(pT)))
    return per_core, mask, ids_restore


_CACHE = {}


def _get_nc(depth):
    if depth not in _CACHE:
        _CACHE[depth] = build_nc(depth)
    return _CACHE[depth]


def run(inputs, depth=12, trace=False):
    nc = _get_nc(depth)
    wts = prep_weights(inputs, depth)
    per_core, mask, ids_restore = prep_percore(inputs)
    in_maps = []
    for c in range(NCORES):
        m = dict(wts)
        m['patches'], m['posT'] = per_core[c]
        in_maps.append(m)
    res = run_bass_kernel_spmd(nc, in_maps, list(range(NCORES)), trace=trace)
    x_vis = np.empty((B, NUM_KEEP, E), np.float32)
    for c in range(NCORES):
        arr = res.results[c]['xvis']                      # [128, EC, T]
        full = arr.transpose(2, 1, 0).reshape(T, E)       # [T, E]
        for i in range(IPC):
            x_vis[c * IPC + i] = full[i * TI:(i + 1) * TI]
    return (x_vis, mask, ids_restore), res


def kernel(**inputs):
    out, _ = run(inputs, depth=12, trace=False)
    return out


# revision 44
# speedup vs baseline: 1.0103x; 1.0103x over previous
"""MAE encoder (ViT-B-ish, 12 layers) on 8 Trainium2 NeuronCores.

Strategy: data-parallel over batch (16 images -> 2 per core), weights
replicated. Inside each core activations live in "feature-on-partition"
layout xT[128, E/128, T] so every projection is matmul(lhsT=W^T, rhs=xT)
with no transposes. Attention computes S^T = k_h^T.T @ q_h^T directly
(keys on partitions), softmax normalizer via a ones-matmul that lands
replicated across partitions, so no transpose is ever needed.
Matmul inputs bf16, residual stream fp32.
"""

import numpy as np
import ml_dtypes

import concourse.bass as bass
import concourse.bacc as bacc
import concourse.mybir as mybir
import concourse.tile as tile
from concourse.bass_utils import run_bass_kernel_spmd

BF16 = mybir.dt.bfloat16
F32 = mybir.dt.float32
F8 = mybir.dt.float8e4
DR = mybir.MatmulPerfMode.DoubleRow
nf8 = mybir.dt.np(F8)
AF = mybir.ActivationFunctionType
OP = mybir.AluOpType

B, C, IMG, P = 16, 1, 512, 16
GRID = IMG // P
N = GRID * GRID          # 1024
E = 768
H = 12
HD = 64
FF = 4 * E               # 3072
NUM_KEEP = N // 2        # 512
LN_EPS = 1e-5
NCORES = 8
IPC = B // NCORES        # images per core = 2
T = IPC * NUM_KEEP       # tokens per core = 1024
TI = NUM_KEEP            # tokens per image = 512
EC = E // 128            # 6 feature chunks
FC = FF // 128           # 24 hidden chunks
PD = C * P * P           # 256 patch dim
KC = PD // 128           # 2

nbf = ml_dtypes.bfloat16

# lnp param tensor column layout (per layer, [128, 72] f32)
LN1W, LN1B, QKB, PRB, LN2W, LN2B, F1B, F2B = 0, 6, 12, 24, 30, 36, 42, 66


def build_nc(depth):
    nc = bacc.Bacc(None, target_bir_lowering=False)
    d = {}
    d['patches'] = nc.dram_tensor('patches', [128, KC, T], BF16, kind='ExternalInput')
    d['posT'] = nc.dram_tensor('posT', [128, EC, T], F32, kind='ExternalInput')
    d['pwT'] = nc.dram_tensor('pwT', [128, KC, E], BF16, kind='ExternalInput')
    d['pepar'] = nc.dram_tensor('pepar', [128, 3 * EC], F32, kind='ExternalInput')
    d['npar'] = nc.dram_tensor('npar', [128, 2 * EC], F32, kind='ExternalInput')
    d['wqk'] = nc.dram_tensor('wqk', [depth, 128, EC, 2 * E], BF16, kind='ExternalInput')
    d['wv'] = nc.dram_tensor('wv', [depth, 128, EC, E], BF16, kind='ExternalInput')
    d['wp'] = nc.dram_tensor('wp', [depth, 128, EC, E], BF16, kind='ExternalInput')
    d['w1'] = nc.dram_tensor('w1', [depth, 2, 128, EC, FF // 2], BF16, kind='ExternalInput')
    d['w2'] = nc.dram_tensor('w2', [depth, 2, 128, FC, E // 2], BF16, kind='ExternalInput')
    d['lnp'] = nc.dram_tensor('lnp', [depth, 128, 72], F32, kind='ExternalInput')
    d['vb'] = nc.dram_tensor('vb', [depth, 128, E], BF16, kind='ExternalInput')
    d['xvis'] = nc.dram_tensor('xvis', [128, EC, T], F32, kind='ExternalOutput')

    with tile.TileContext(nc) as tc:
        _emit(nc, tc, d, depth)
    nc.finalize()
    return nc


PHASE_MARKS = []


def _mark(nc, label):
    n = nc.next_id()   # consumes one id; fine, just a marker
    PHASE_MARKS.append((n, label))


def _emit(nc, tc, d, depth):
    PHASE_MARKS.clear()
    import contextlib
    ctx = contextlib.ExitStack()
    with ctx:
        wpool = ctx.enter_context(tc.tile_pool(name='w', bufs=2))
        state = ctx.enter_context(tc.tile_pool(name='state', bufs=1))
        hx = ctx.enter_context(tc.tile_pool(name='hx', bufs=2))
        actp = ctx.enter_context(tc.tile_pool(name='act', bufs=2))
        actg = ctx.enter_context(tc.tile_pool(name='actg', bufs=1))
        atp = ctx.enter_context(tc.tile_pool(name='at', bufs=3))
        stp = ctx.enter_context(tc.tile_pool(name='st', bufs=2))
        tmpp = ctx.enter_context(tc.tile_pool(name='tmp', bufs=2))
        smallp = ctx.enter_context(tc.tile_pool(name='small', bufs=2))
        vbp = ctx.enter_context(tc.tile_pool(name='vbp', bufs=1))
        outp = ctx.enter_context(tc.tile_pool(name='outp', bufs=1))
        ps = ctx.enter_context(tc.tile_pool(name='ps', bufs=6, space='PSUM'))
        pstat = ctx.enter_context(tc.tile_pool(name='pstat', bufs=2, space='PSUM'))

        # persistent state
        xres = state.tile([128, EC, T], F32)       # residual stream (both imgs)
        ones = state.tile([128, 128], BF16)
        nc.vector.memset(ones, 1.0)
        epst = state.tile([128, 1], F32)
        nc.vector.memset(epst, LN_EPS)

        def mm_chain(out_ap, lhsT_slices, rhs_slices, tile_position=None):
            n = len(lhsT_slices)
            for i in range(n):
                nc.tensor.matmul(out_ap, lhsT_slices[i], rhs_slices[i],
                                 start=(i == 0), stop=(i == n - 1),
                                 tile_position=tile_position)

        def ln_stats(xb, nchunks):
            """xb: [128, nchunks, TI] bf16. Returns (m_sb, rstd) [128, TI] f32."""
            s_ps = pstat.tile([128, TI], F32, tag='stat')
            sq_ps = pstat.tile([128, TI], F32, tag='stat')
            mm_chain(s_ps, [ones] * nchunks,
                     [xb[:, k, :] for k in range(nchunks)])
            for k in range(nchunks):
                xsq = tmpp.tile([128, TI], BF16, tag='xsq')
                nc.vector.tensor_mul(xsq, xb[:, k, :], xb[:, k, :])
                nc.tensor.matmul(sq_ps, ones, xsq, start=(k == 0), stop=(k == nchunks - 1))
            cinv = 1.0 / (nchunks * 128)
            m_sb = stp.tile([128, TI], F32, tag='m')
            nc.vector.tensor_scalar_mul(m_sb, s_ps, cinv)
            m2 = stp.tile([128, TI], F32, tag='m2')
            nc.vector.tensor_mul(m2, m_sb, m_sb)
            var = stp.tile([128, TI], F32, tag='var')
            nc.vector.scalar_tensor_tensor(var, sq_ps, cinv, m2,
                                           op0=OP.mult, op1=OP.subtract)
            # rstd = exp(-0.5*ln(var+eps)); Ln+Exp share one ACT table set
            nc.scalar.activation(var, var, AF.Ln, bias=epst)
            nc.scalar.activation(var, var, AF.Exp, scale=-0.5)
            return m_sb, var

        def ln_apply(xb, m_sb, rstd, w_col, b_col, out_tile, nchunks):
            """out[:,k,:] = ((xb[:,k,:]-m)*rstd)*w[k] + b[k]"""
            for k in range(nchunks):
                t0 = tmpp.tile([128, TI], BF16, tag='ta')
                nc.vector.tensor_sub(t0, xb[:, k, :], m_sb)
                t1 = tmpp.tile([128, TI], BF16, tag='tb')
                nc.vector.tensor_mul(t1, t0, rstd)
                nc.scalar.activation(out_tile[:, k, :], t1, AF.Identity,
                                     bias=b_col(k), scale=w_col(k))

        # ---------------- patch embed + pe-norm + pos ----------------
        _mark(nc, 'patch')
        pw = wpool.tile([128, KC, E], BF16, tag='w')
        nc.sync.dma_start(pw, d['pwT'][:])
        pep = smallp.tile([128, 3 * EC], F32, tag='pep')
        nc.sync.dma_start(pep, d['pepar'][:])
        for img in range(IPC):
            pt = wpool.tile([128, KC, TI], BF16, tag='w')
            nc.sync.dma_start(pt, d['patches'][:, :, img * TI:(img + 1) * TI])
            tb = hx.tile([128, EC, TI], BF16, tag='hx')  # embedded tokens bf16
            for ec in range(EC):
                pp = ps.tile([128, TI], F32, tag='mm')
                mm_chain(pp, [pw[:, k, ec * 128:(ec + 1) * 128] for k in range(KC)],
                         [pt[:, k, :] for k in range(KC)])
                nc.scalar.activation(tb[:, ec, :], pp, AF.Identity,
                                     bias=pep[:, ec:ec + 1])
            m_sb, rstd = ln_stats(tb, EC)
            for ec in range(EC):
                t0 = tmpp.tile([128, TI], BF16, tag='ta')
                nc.vector.tensor_sub(t0, tb[:, ec, :], m_sb)
                t1 = tmpp.tile([128, TI], BF16, tag='tb')
                nc.vector.tensor_mul(t1, t0, rstd)
                t2 = tmpp.tile([128, TI], F32, tag='tc')
                nc.scalar.activation(t2, t1, AF.Identity,
                                     bias=pep[:, 2 * EC + ec:2 * EC + ec + 1],
                                     scale=pep[:, EC + ec:EC + ec + 1])
                pos = outp.tile([128, TI], F32, tag='o')
                nc.sync.dma_start(pos, d['posT'][:, ec, img * TI:(img + 1) * TI])
                nc.vector.tensor_add(xres[:, ec, img * TI:(img + 1) * TI], t2, pos)

        # ---------------- transformer layers ----------------
        for l in range(depth):
            lnp = smallp.tile([128, 72], F32, tag='lnp')
            nc.sync.dma_start(lnp, d['lnp'][l])
            vb = vbp.tile([128, E], BF16, tag='vb')
            nc.sync.dma_start(vb, d['vb'][l])
            wqk = wpool.tile([128, EC, 2 * E], BF16, tag='w')
            nc.sync.dma_start(wqk, d['wqk'][l])
            wv = wpool.tile([128, EC, E], BF16, tag='w')
            nc.sync.dma_start(wv, d['wv'][l])

            qks, vs, ots = [], [], []
            # ---- phase A: LN1 + qkv (both images) ----
            for img in range(IPC):
                tsl = slice(img * TI, (img + 1) * TI)
                # ---- LN1 -> h ----
                _mark(nc, 'ln1')
                xb = hx.tile([128, EC, TI], BF16, tag='hx')
                for k in range(EC):
                    nc.vector.tensor_copy(xb[:, k, :], xres[:, k, tsl])
                m_sb, rstd = ln_stats(xb, EC)
                h = hx.tile([128, EC, TI], BF16, tag='hx')
                ln_apply(xb, m_sb, rstd,
                         lambda k: lnp[:, LN1W + k:LN1W + k + 1],
                         lambda k: lnp[:, LN1B + k:LN1B + k + 1], h, EC)

                # ---- qkv ----
                _mark(nc, 'qkv')
                qk = actp.tile([128, 2 * EC, TI], BF16, tag='qk')
                for mc in range(2 * EC):
                    pp = ps.tile([128, TI], F32, tag='mm')
                    mm_chain(pp, [wqk[:, k, mc * 128:(mc + 1) * 128] for k in range(EC)],
                             [h[:, k, :] for k in range(EC)])
                    nc.scalar.activation(qk[:, mc, :], pp, AF.Identity,
                                         bias=lnp[:, QKB + mc:QKB + mc + 1])
                # per-head 128-col slots: [v_h(64) | ones(64)] so the PV chain
                # computes O (rows 0:64) and the softmax denom (rows 64:128)
                v = actp.tile([128, TI // 128, 2 * E], BF16, tag='v')
                ones_ap = bass.AP(tensor=v.tensor, offset=v.offset + 64,
                                  ap=[v.ap[0], [2 * E, TI // 128], [128, H], [1, 64]])
                nc.vector.memset(ones_ap, 1.0)
                for tcx in range(TI // 128):
                    for nh in range(2):
                        nsl = slice(nh * (E // 2), (nh + 1) * (E // 2))
                        pp = ps.tile([128, E // 2], F32, tag='mm')
                        mm_chain(pp, [h[:, k, tcx * 128:(tcx + 1) * 128] for k in range(EC)],
                                 [wv[:, k, nsl] for k in range(EC)])
                        dst = bass.AP(tensor=v.tensor,
                                      offset=v.offset + tcx * 2 * E + nh * 6 * 128,
                                      ap=[v.ap[0], [128, 6], [1, 64]])
                        nc.vector.tensor_tensor(
                            dst, pp.rearrange('p (hh c) -> p hh c', c=64),
                            vb[:, nsl].rearrange('p (hh c) -> p hh c', c=64), OP.add)
                qks.append(qk)
                vs.append(v)

            # ---- phase B: attention (both images) ----
            _mark(nc, 'attn')
            for img in range(IPC):
                qk, v = qks[img], vs[img]
                ot = actp.tile([128, EC, TI], BF16, tag='ot')
                for hd in range(H):
                    qc, ro = hd // 2, 64 * (hd % 2)
                    at = atp.tile([128, 4, TI], BF16, tag='at')
                    for jc in range(4):
                        sp = ps.tile([128, TI], F32, tag='mm')
                        nc.tensor.matmul(
                            sp,
                            qk[ro:ro + 64, EC + qc, jc * 128:(jc + 1) * 128],
                            qk[ro:ro + 64, qc, :],
                            start=True, stop=True)
                        nc.scalar.activation(at[:, jc, :], sp, AF.Exp)
                    op_t = ps.tile([128, TI], F32, tag='mm')
                    for jc in range(4):
                        nc.tensor.matmul(op_t,
                                         v[:, jc, hd * 128:hd * 128 + 128],
                                         at[:, jc, :],
                                         start=(jc == 0), stop=(jc == 3))
                    r = stp.tile([64, TI], F32, tag='r')
                    nc.vector.reciprocal(r, op_t[64:128, :])
                    nc.vector.tensor_mul(ot[ro:ro + 64, qc, :],
                                         op_t[0:64, :], r)
                ots.append(ot)

            # ---- phase C: proj + residual + LN2 (both images) ----
            wp = wpool.tile([128, EC, E], BF16, tag='w')
            nc.sync.dma_start(wp, d['wp'][l])
            h2s = []
            for img in range(IPC):
                tsl = slice(img * TI, (img + 1) * TI)
                _mark(nc, 'proj')
                ot = ots[img]
                for ec in range(EC):
                    pp = ps.tile([128, TI], F32, tag='mm')
                    mm_chain(pp, [wp[:, k, ec * 128:(ec + 1) * 128] for k in range(EC)],
                             [ot[:, k, :] for k in range(EC)])
                    nc.vector.scalar_tensor_tensor(
                        xres[:, ec, tsl], pp, lnp[:, PRB + ec:PRB + ec + 1],
                        xres[:, ec, tsl], op0=OP.add, op1=OP.add)
                _mark(nc, 'ln2')
                xb2 = hx.tile([128, EC, TI], BF16, tag='hx')
                for k in range(EC):
                    nc.vector.tensor_copy(xb2[:, k, :], xres[:, k, tsl])
                m_sb2, rstd2 = ln_stats(xb2, EC)
                h2 = hx.tile([128, EC, TI], BF16, tag='hx')
                ln_apply(xb2, m_sb2, rstd2,
                         lambda k: lnp[:, LN2W + k:LN2W + k + 1],
                         lambda k: lnp[:, LN2B + k:LN2B + k + 1], h2, EC)
                h2s.append(h2)

            # ---- phase D: MLP (both images) ----
            _mark(nc, 'mlp')
            for img in range(IPC):
                tsl = slice(img * TI, (img + 1) * TI)
                h2 = h2s[img]
                g = actg.tile([128, FC, TI], BF16, tag='g')
                for half in range(2):
                    w1h = wpool.tile([128, EC, FF // 2], BF16, tag='w')
                    nc.sync.dma_start(w1h, d['w1'][l, half])
                    for hc in range(FC // 2):
                        ghc = half * (FC // 2) + hc
                        pp = ps.tile([128, TI], F32, tag='mm')
                        mm_chain(pp, [w1h[:, k, hc * 128:(hc + 1) * 128] for k in range(EC)],
                                 [h2[:, k, :] for k in range(EC)])
                        nc.scalar.activation(g[:, ghc, :], pp, AF.Gelu,
                                             bias=lnp[:, F1B + ghc:F1B + ghc + 1])
                for half in range(2):
                    w2h = wpool.tile([128, FC, E // 2], BF16, tag='w')
                    nc.sync.dma_start(w2h, d['w2'][l, half])
                    for ec in range(EC // 2):
                        oec = half * (EC // 2) + ec
                        pp = ps.tile([128, TI], F32, tag='mm')
                        mm_chain(pp, [w2h[:, k, ec * 128:(ec + 1) * 128] for k in range(FC)],
                                 [g[:, k, :] for k in range(FC)])
                        nc.vector.scalar_tensor_tensor(
                            xres[:, oec, tsl], pp, lnp[:, F2B + oec:F2B + oec + 1],
                            xres[:, oec, tsl], op0=OP.add, op1=OP.add)

        # ---------------- final norm + output ----------------
        _mark(nc, 'final')
        npar = smallp.tile([128, 2 * EC], F32, tag='npar')
        nc.sync.dma_start(npar, d['npar'][:])
        for img in range(IPC):
            tsl = slice(img * TI, (img + 1) * TI)
            xb = hx.tile([128, EC, TI], BF16, tag='hx')
            for k in range(EC):
                nc.vector.tensor_copy(xb[:, k, :], xres[:, k, tsl])
            m_sb, rstd = ln_stats(xb, EC)
            for ec in range(EC):
                t0 = tmpp.tile([128, TI], F32, tag='fa')
                nc.vector.tensor_sub(t0, xres[:, ec, tsl], m_sb)
                t1 = tmpp.tile([128, TI], F32, tag='fb')
                nc.vector.tensor_mul(t1, t0, rstd)
                o = outp.tile([128, TI], F32, tag='o')
                nc.scalar.activation(o, t1, AF.Identity,
                                     bias=npar[:, EC + ec:EC + ec + 1],
                                     scale=npar[:, ec:ec + 1])
                nc.sync.dma_start(d['xvis'][:, ec, tsl], o)


# ---------------------------------------------------------------------------
# host-side data prep
# ---------------------------------------------------------------------------

def _featT(W):
    """W [O, I] row-major -> [128, I//128, O] so tile[ki, ks, o] = W[o, ks*128+ki]."""
    O, I = W.shape
    return np.ascontiguousarray(W.T.reshape(I // 128, 128, O).transpose(1, 0, 2))


def _tokT(X):
    """X [T, E] -> [128, E//128, T] so tile[ki, ks, t] = X[t, ks*128+ki]."""
    Tn, En = X.shape
    return np.ascontiguousarray(X.T.reshape(En // 128, 128, Tn).transpose(1, 0, 2))


def _cols(vec):
    """[D] -> [128, D//128] so out[ki, k] = vec[k*128+ki]."""
    D = vec.shape[0]
    return np.ascontiguousarray(vec.reshape(D // 128, 128).T)


def prep_weights(inputs, depth):
    f32 = np.float32
    qkv_w = np.asarray(inputs['qkv_w'], f32).copy()
    qkv_b = np.asarray(inputs['qkv_b'], f32).copy()
    scale = HD ** -0.5
    qkv_w[:, :E, :] *= scale
    qkv_b[:, :E] *= scale

    wqk = np.stack([_featT(qkv_w[l, :2 * E, :]) for l in range(depth)]).astype(nbf)
    # v weights as rhs: [ki, ks, d] = Wv[d, ks*128+ki] = Wv.T[ks*128+ki, d]
    wv = np.stack([_featT(qkv_w[l, 2 * E:, :]) for l in range(depth)]).astype(nbf)
    wp = np.stack([_featT(np.asarray(inputs['proj_w'][l], f32)) for l in range(depth)]).astype(nbf)
    w1full = [_featT(np.asarray(inputs['fc1_w'][l], f32)) for l in range(depth)]
    w1 = np.stack([np.stack([w[:, :, :FF // 2], w[:, :, FF // 2:]]) for w in w1full]).astype(nbf)
    w2full = [_featT(np.asarray(inputs['fc2_w'][l], f32)) for l in range(depth)]
    w2 = np.stack([np.stack([w[:, :, :E // 2], w[:, :, E // 2:]]) for w in w2full]).astype(nbf)

    lnp = np.zeros((depth, 128, 72), f32)
    for l in range(depth):
        lnp[l, :, LN1W:LN1W + 6] = _cols(np.asarray(inputs['ln1_w'][l], f32))
        lnp[l, :, LN1B:LN1B + 6] = _cols(np.asarray(inputs['ln1_b'][l], f32))
        lnp[l, :, QKB:QKB + 12] = _cols(qkv_b[l, :2 * E])
        lnp[l, :, PRB:PRB + 6] = _cols(np.asarray(inputs['proj_b'][l], f32))
        lnp[l, :, LN2W:LN2W + 6] = _cols(np.asarray(inputs['ln2_w'][l], f32))
        lnp[l, :, LN2B:LN2B + 6] = _cols(np.asarray(inputs['ln2_b'][l], f32))
        lnp[l, :, F1B:F1B + 24] = _cols(np.asarray(inputs['fc1_b'][l], f32))
        lnp[l, :, F2B:F2B + 6] = _cols(np.asarray(inputs['fc2_b'][l], f32))
    vbr = np.broadcast_to(qkv_b[:depth, 2 * E:, None].transpose(0, 2, 1),
                          (depth, 128, E)).astype(nbf)
    vbr = np.ascontiguousarray(vbr)

    pwT = _featT(np.asarray(inputs['patch_w'], f32).reshape(E, PD)).astype(nbf)
    pepar = np.zeros((128, 18), f32)
    pepar[:, 0:6] = _cols(np.asarray(inputs['patch_b'], f32))
    pepar[:, 6:12] = _cols(np.asarray(inputs['pe_norm_w'], f32))
    pepar[:, 12:18] = _cols(np.asarray(inputs['pe_norm_b'], f32))
    npar = np.zeros((128, 12), f32)
    npar[:, 0:6] = _cols(np.asarray(inputs['norm_w'], f32))
    npar[:, 6:12] = _cols(np.asarray(inputs['norm_b'], f32))
    return dict(wqk=wqk, wv=wv, wp=wp, w1=w1, w2=w2, lnp=lnp, vb=vbr,
                pwT=pwT, pepar=pepar, npar=npar)


def prep_percore(inputs):
    f32 = np.float32
    x = np.asarray(inputs['x'], f32)
    noise = np.asarray(inputs['noise'], f32)
    ids_shuffle = np.argsort(noise, axis=1, kind='stable').astype(np.int32)
    ids_restore = np.argsort(ids_shuffle, axis=1, kind='stable').astype(np.int32)
    ids_keep = ids_shuffle[:, :NUM_KEEP]
    mask = np.ones((B, N), f32)
    mask[:, :NUM_KEEP] = 0.0
    mask = np.take_along_axis(mask, ids_restore, axis=1)

    patches = x.reshape(B, GRID, P, GRID, P).transpose(0, 1, 3, 2, 4).reshape(B, N, PD)
    pos = np.asarray(inputs['pos_embed'], f32)[0]        # [N, E]

    per_core = []
    for c in range(NCORES):
        sel = ids_keep[c * IPC:(c + 1) * IPC]            # [IPC, 512]
        pk = np.concatenate([patches[c * IPC + i][sel[i]] for i in range(IPC)])  # [T, PD]
        pc = _tokT(pk).astype(nbf)                        # [128, KC, T]
        pg = np.concatenate([pos[sel[i]] for i in range(IPC)])                   # [T, E]
        pT = _tokT(pg)                                    # [128, EC, T] f32
        per_core.append((np.ascontiguousarray(pc), np.ascontiguousarray(pT)))
    return per_core, mask, ids_restore


_CACHE = {}


def _get_nc(depth):
    if depth not in _CACHE:
        _CACHE[depth] = build_nc(depth)
    return _CACHE[depth]


def run(inputs, depth=12, trace=False):
    nc = _get_nc(depth)
    wts = prep_weights(inputs, depth)
    per_core, mask, ids_restore = prep_percore(inputs)
    in_maps = []
    for c in range(NCORES):
        m = dict(wts)
        m['patches'], m['posT'] = per_core[c]
        in_maps.append(m)
    res = run_bass_kernel_spmd(nc, in_maps, list(range(NCORES)), trace=trace)
    x_vis = np.empty((B, NUM_KEEP, E), np.float32)
    for c in range(NCORES):
        arr = res.results[c]['xvis']                      # [128, EC, T]
        full = arr.transpose(2, 1, 0).reshape(T, E)       # [T, E]
        for i in range(IPC):
            x_vis[c * IPC + i] = full[i * TI:(i + 1) * TI]
    return (x_vis, mask, ids_restore), res


def kernel(**inputs):
    out, _ = run(inputs, depth=12, trace=False)
    return out
